# revision 1
# baseline (speedup 1.0000x reference)
"""BiLSTM-CRF loss for nn_BiLSTM_CRF_68152541053203 on 8 TRN2 NeuronCores.

Sharding: data-parallel over batch (B=64 -> 8 rows/core). Each core runs a
Bass kernel computing the word-BiLSTM input projections for its batch shard:
    xg[2048, 2048] = x_shard[2048, 320] @ [Wih_f.T | Wih_b.T](320, 2048)
(the dominant dense FLOPs). The strictly-sequential LSTM recurrences and the
tiny CRF scan run on host fp32, matching the reference step-for-step.
"""

import numpy as np

import concourse.bacc as bacc
import concourse.mybir as mybir
import concourse.tile as tile
from concourse.bass_utils import run_bass_kernel_spmd

N_CORES = 8
B, T = 64, 256
CIN, CH = 25, 10
EMB_IN, H = 320, 256
K = 20
BL = B // N_CORES          # 8 rows per core
M_ROWS = BL * T            # 2048
KDIM = EMB_IN              # 320 contraction
NCOLS = 2 * 4 * H          # 2048 = fwd(1024) | bwd(1024)

_CACHE = {}


def _build_nc():
    nc = bacc.Bacc("TRN2", target_bir_lowering=False, debug=False,
                   num_devices=N_CORES)
    xT = nc.dram_tensor("xT", [KDIM, M_ROWS], mybir.dt.float32r,
                        kind="ExternalInput").ap()
    w = nc.dram_tensor("w", [KDIM, NCOLS], mybir.dt.float32r,
                       kind="ExternalInput").ap()
    xg = nc.dram_tensor("xg", [M_ROWS, NCOLS], mybir.dt.float32,
                        kind="ExternalOutput").ap()

    KT = [(0, 128), (128, 128), (256, 64)]      # k-tiles of 320
    NT = 512                                     # psum-bank limit fp32
    with tile.TileContext(nc) as tc:
        with (
            tc.tile_pool(name="wx", bufs=1) as wx,
            tc.tile_pool(name="ps", bufs=8, space="PSUM") as ps,
            tc.tile_pool(name="ot", bufs=4) as ot,
        ):
            wk, xk = [], []
            for i, (k0, kn) in enumerate(KT):
                wt = wx.tile([kn, NCOLS], mybir.dt.float32r, tag=f"w{i}")
                nc.gpsimd.dma_start(wt[:], w[k0:k0 + kn, :])
                wk.append(wt)
                xt = wx.tile([kn, M_ROWS], mybir.dt.float32r, tag=f"x{i}")
                nc.gpsimd.dma_start(xt[:], xT[k0:k0 + kn, :])
                xk.append(xt)
            for m in range(M_ROWS // 128):
                o = ot.tile([128, NCOLS], mybir.dt.float32)
                for n in range(NCOLS // NT):
                    acc = ps.tile([128, NT], mybir.dt.float32)
                    for i in range(len(KT)):
                        nc.tensor.matmul(
                            acc[:],
                            xk[i][:, m * 128:(m + 1) * 128],
                            wk[i][:, n * NT:(n + 1) * NT],
                            start=(i == 0), stop=(i == len(KT) - 1),
                        )
                    osl = o[:, n * NT:(n + 1) * NT]
                    if n % 2 == 0:
                        nc.vector.tensor_copy(osl, acc[:])
                    else:
                        nc.scalar.copy(osl, acc[:])
                nc.gpsimd.dma_start(xg[m * 128:(m + 1) * 128, :], o[:])
    nc.compile()
    return nc


def _sigmoid(x):
    return 1.0 / (1.0 + np.exp(-x))


def _lstm_dir_from_xg(xg, Whh):
    """xg: (B,T,4H) bias-included input projections. Returns (B,T,H) fp32."""
    Bs, Ts, G = xg.shape
    Hd = G // 4
    WhhT = np.ascontiguousarray(Whh.T)
    h = np.zeros((Bs, Hd), np.float32)
    c = np.zeros((Bs, Hd), np.float32)
    out = np.empty((Bs, Ts, Hd), np.float32)
    for t in range(Ts):
        g = xg[:, t] + h @ WhhT
        i = _sigmoid(g[:, :Hd])
        f = _sigmoid(g[:, Hd:2 * Hd])
        gg = np.tanh(g[:, 2 * Hd:3 * Hd])
        o = _sigmoid(g[:, 3 * Hd:])
        c = f * c + i * gg
        h = o * np.tanh(c)
        out[:, t] = h
    return out


def _lstm_dir_host(x, Wih, Whh, b):
    xg = np.einsum('bti,gi->btg', x, Wih, optimize=True) + b
    return _lstm_dir_from_xg(xg.astype(np.float32), Whh)


def _logsumexp(a, axis):
    m = np.max(a, axis=axis, keepdims=True)
    return (m + np.log(np.sum(np.exp(a - m), axis=axis, keepdims=True))).squeeze(axis)


def kernel(char_tensor, token_tensor, tags, mask, emb,
           cWih_f, cWhh_f, cb_f, cWih_b, cWhh_b, cb_b,
           wWih_f, wWhh_f, wb_f, wWih_b, wWhh_b, wb_b,
           Wtag, btag, start_t, end_t, trans):
    f32 = lambda a: np.asarray(a, np.float32)
    char_tensor = f32(char_tensor)
    emb = f32(emb)
    token_tensor = np.asarray(token_tensor).astype(np.int64)
    tags_i = np.asarray(tags).astype(np.int64)
    mask_b = np.asarray(mask).astype(bool)

    # --- char BiLSTM (tiny) + embedding gather on host ---
    cf = _lstm_dir_host(char_tensor, f32(cWih_f), f32(cWhh_f), f32(cb_f))
    cb = _lstm_dir_host(char_tensor[:, ::-1], f32(cWih_b), f32(cWhh_b),
                        f32(cb_b))[:, ::-1]
    word_emb = emb[token_tensor]                                  # (B,T,300)
    x = np.concatenate([cf, cb, word_emb], axis=2)                # (B,T,320)

    # --- word-LSTM input projections on the 8 NeuronCores ---
    if "nc" not in _CACHE:
        _CACHE["nc"] = _build_nc()
    nc = _CACHE["nc"]
    w_cat = np.ascontiguousarray(
        np.concatenate([f32(wWih_f).T, f32(wWih_b).T], axis=1))   # (320,2048)
    in_maps = []
    for ci in range(N_CORES):
        xs = x[ci * BL:(ci + 1) * BL].reshape(M_ROWS, KDIM)
        in_maps.append({"xT": np.ascontiguousarray(xs.T), "w": w_cat})
    _CACHE["last_in_maps"] = in_maps
    # First exec on a freshly-compiled NEFF occasionally hits a transient
    # NRT_EXEC_UNIT_UNRECOVERABLE on this axon tunnel; a retry (with a fresh
    # build on the second failure) has always succeeded.
    res = None
    for attempt in range(3):
        try:
            res = run_bass_kernel_spmd(nc, in_maps,
                                       core_ids=list(range(N_CORES)))
            break
        except Exception:
            if attempt == 2:
                raise
            import time as _time
            _time.sleep(5)
            if attempt == 1:
                _CACHE.pop("nc", None)
                nc = _CACHE.setdefault("nc", _build_nc())
    xg_all = np.concatenate(
        [r["xg"].reshape(BL, T, NCOLS) for r in res.results], axis=0)
    xg_f = xg_all[:, :, :4 * H] + f32(wb_f)
    xg_b = xg_all[:, :, 4 * H:] + f32(wb_b)

    # --- word BiLSTM recurrence (sequential, host) ---
    hf = _lstm_dir_from_xg(xg_f, f32(wWhh_f))
    hb = _lstm_dir_from_xg(xg_b[:, ::-1], f32(wWhh_b))[:, ::-1]
    seq = np.concatenate([hf, hb], axis=2)                        # (B,T,512)

    # --- emissions + CRF NLL ---
    em = np.einsum('bth,kh->btk', seq, f32(Wtag), optimize=True) + f32(btag)
    em = np.swapaxes(em, 0, 1)                                    # (T,B,K)
    tg = np.swapaxes(tags_i, 0, 1)
    m = np.swapaxes(mask_b, 0, 1).astype(np.float32)
    start_t, end_t, trans = f32(start_t), f32(end_t), f32(trans)
    bidx = np.arange(B)
    e_sc = np.take_along_axis(em, tg[..., None], axis=-1)[..., 0]  # (T,B)
    num = start_t[tg[0]] + e_sc[0]
    num = num + np.sum((trans[tg[:-1], tg[1:]] + e_sc[1:]) * m[1:], axis=0)
    last = (np.sum(m, axis=0) - 1).astype(np.int64)
    num = num + end_t[tg[last, bidx]]
    alpha = start_t[None, :] + em[0]
    for t in range(1, T):
        nxt = _logsumexp(alpha[:, :, None] + trans[None, :, :]
                         + em[t][:, None, :], axis=1)
        alpha = np.where(m[t][:, None] > 0, nxt, alpha)
    den = _logsumexp(alpha + end_t[None, :], axis=1)
    return np.float32(-np.sum(num - den))



# revision 2
# speedup vs baseline: 1.9603x; 1.9603x over previous
"""BiLSTM-CRF loss for nn_BiLSTM_CRF_68152541053203 on 8 TRN2 NeuronCores.

Sharding: data-parallel over batch (B=64 -> BL=8 rows/core). Each core runs
the full word-BiLSTM on its batch shard entirely on-device:
  - input projections xgT = Wih_aug @ x_augT (LSTM bias folded via an
    appended ones-column on x),
  - the 256-step forward+backward LSTM recurrences,
  - emissions emT = Wtag @ seqT.
Device layout keeps hidden/gate dims on SBUF partitions and batch on the
free dim, with bf16 matmul operands (fp32 cell state), so the sequential
recurrence is TensorE-bound instead of ACT/DVE-bound. Only the (20, 2048)
emission logits per core return to the host (~1.3 MB total instead of the
134 MB of gate pre-activations a host-recurrence design needs), where the
cheap char-BiLSTM, embedding gather and CRF run in numpy fp32.
"""

import numpy as np
import ml_dtypes

import concourse.bacc as bacc
import concourse.mybir as mybir
import concourse.tile as tile
from concourse.bass_utils import run_bass_kernel_spmd

BF16 = ml_dtypes.bfloat16
F32 = np.float32

N_CORES = 8
B, T = 64, 256
CIN, CH = 25, 10
EMB_IN, H = 320, 256
K = 20
BL = B // N_CORES          # 8 batch rows per core
ROWS = BL * T              # 2048
KIN = EMB_IN + 1           # ones column folds the LSTM input bias
G = 4 * H                  # 1024 gates per direction
KT = [(0, 128), (128, 128), (256, KIN - 256)]
HC = BL * (T + 1)          # h columns per k-section incl. zero pad
ACT = mybir.ActivationFunctionType

# PyTorch gate rows [i, f, g, o] -> device tile order [i, f, o, g]
PERM = np.r_[0:H, H:2 * H, 3 * H:4 * H, 2 * H:3 * H]

_CACHE = {}


def _build_nc():
    nc = bacc.Bacc("TRN2", target_bir_lowering=False, debug=False,
                   num_devices=N_CORES)
    xT = nc.dram_tensor("xT", [KIN, ROWS], mybir.dt.bfloat16,
                        kind="ExternalInput").ap()
    wihT = nc.dram_tensor("wihT", [KIN, 2 * G], mybir.dt.bfloat16,
                          kind="ExternalInput").ap()
    whhT = nc.dram_tensor("whhT", [H, 2 * G], mybir.dt.bfloat16,
                          kind="ExternalInput").ap()
    wtagT = nc.dram_tensor("wtagT", [2 * H, K], mybir.dt.bfloat16,
                           kind="ExternalInput").ap()
    emT = nc.dram_tensor("emT", [K, ROWS], mybir.dt.float32,
                         kind="ExternalOutput").ap()

    with tile.TileContext(nc) as tc:
        with (
            tc.tile_pool(name="w", bufs=1) as wp,
            tc.tile_pool(name="psX", bufs=3, space="PSUM") as psX,
            tc.tile_pool(name="psR", bufs=3, space="PSUM") as psR,
            tc.tile_pool(name="psE", bufs=2, space="PSUM") as psE,
            tc.tile_pool(name="gact", bufs=4) as gact,
            tc.tile_pool(name="tmp", bufs=6) as tmpp,
        ):
            xk, wih = [], []
            for i, (k0, kn) in enumerate(KT):
                t = wp.tile([kn, ROWS], mybir.dt.bfloat16, tag=f"xk{i}")
                nc.gpsimd.dma_start(t[:], xT[k0:k0 + kn, :])
                xk.append(t)
                t = wp.tile([kn, 2 * G], mybir.dt.bfloat16, tag=f"wih{i}")
                nc.gpsimd.dma_start(t[:], wihT[k0:k0 + kn, :])
                wih.append(t)
            whh = []
            for i in range(2):
                t = wp.tile([128, 2 * G], mybir.dt.bfloat16, tag=f"whh{i}")
                nc.gpsimd.dma_start(t[:], whhT[128 * i:128 * (i + 1), :])
                whh.append(t)
            wtag = []
            for i in range(4):
                t = wp.tile([128, K], mybir.dt.bfloat16, tag=f"wtag{i}")
                nc.gpsimd.dma_start(t[:], wtagT[128 * i:128 * (i + 1), :])
                wtag.append(t)

            # xgT mega-tile: free layout (t, gate-tile m 0..15, batch b);
            # m 0..7 = fwd tiles [i0 i1 f0 f1 o0 o1 g0 g1], m 8..15 = bwd.
            XG = wp.tile([128, T * 128], mybir.dt.bfloat16, tag="XG")
            XGr = XG[:].rearrange("p (t m b) -> p t m b", t=T, m=16, b=BL)
            # h sequences, bf16, one tile per dir; free layout (k-section, col)
            # fwd: h_t at col 8*(t+1) (zeros at 0:8); bwd: h_t at col 8*t
            # (zeros at 8T:8(T+1)).
            HF = wp.tile([128, 2 * HC], mybir.dt.bfloat16, tag="HF")
            HB = wp.tile([128, 2 * HC], mybir.dt.bfloat16, tag="HB")
            HFr = HF[:].rearrange("p (k c) -> p k c", k=2)
            HBr = HB[:].rearrange("p (k c) -> p k c", k=2)
            nc.vector.memset(HFr[:, :, 0:BL], 0.0)
            nc.vector.memset(HBr[:, :, BL * T:BL * (T + 1)], 0.0)
            CF = wp.tile([128, 16], mybir.dt.float32, tag="CF")
            CB = wp.tile([128, 16], mybir.dt.float32, tag="CB")
            nc.vector.memset(CF[:], 0.0)
            nc.vector.memset(CB[:], 0.0)

            # ---- input projections ----
            NCH = ROWS // 512
            chunk_order = [0, NCH - 1] + list(range(1, NCH - 1))
            for m in range(16):
                for c in chunk_order:
                    ps = psX.tile([128, 512], mybir.dt.float32)
                    for ki, (k0, kn) in enumerate(KT):
                        nc.tensor.matmul(
                            ps[:],
                            wih[ki][:, 128 * m:128 * (m + 1)],
                            xk[ki][:, 512 * c:512 * (c + 1)],
                            start=(ki == 0), stop=(ki == len(KT) - 1),
                        )
                    tpc = 512 // BL
                    nc.vector.tensor_copy(
                        XGr[:, tpc * c:tpc * (c + 1), m, :],
                        ps[:].rearrange("p (t b) -> p t b", b=BL),
                    )

            # ---- recurrence (fwd and bwd as independent chains) ----
            for t in range(T):
                for d in range(2):
                    td = t if d == 0 else T - 1 - t
                    Hr = HFr if d == 0 else HBr
                    Cst = CF if d == 0 else CB
                    src = BL * td if d == 0 else BL * (td + 1)
                    dst = BL * (td + 1) if d == 0 else BL * td
                    ps = psR.tile([128, 64], mybir.dt.float32)
                    for g in range(8):
                        for k in range(2):
                            nc.tensor.matmul(
                                ps[:, 8 * g:8 * (g + 1)],
                                whh[k][:, G * d + 128 * g:
                                       G * d + 128 * (g + 1)],
                                Hr[:, k, src:src + BL],
                                start=(k == 0), stop=(k == 1),
                            )
                    S = gact.tile([128, 64], mybir.dt.float32)
                    nc.vector.tensor_add(
                        S[:].rearrange("p (m b) -> p m b", b=BL),
                        ps[:].rearrange("p (m b) -> p m b", b=BL),
                        XGr[:, td, 8 * d:8 * (d + 1), :],
                    )
                    nc.scalar.activation(S[:, 0:48], S[:, 0:48], ACT.Sigmoid)
                    nc.scalar.activation(S[:, 48:64], S[:, 48:64], ACT.Tanh)
                    t1 = tmpp.tile([128, 16], mybir.dt.float32, tag="t1")
                    t2 = tmpp.tile([128, 16], mybir.dt.float32, tag="t2")
                    nc.vector.tensor_mul(t1[:], S[:, 0:16], S[:, 48:64])
                    nc.vector.tensor_mul(t2[:], S[:, 16:32], Cst[:])
                    nc.vector.tensor_add(Cst[:], t1[:], t2[:])
                    th = tmpp.tile([128, 16], mybir.dt.float32, tag="th")
                    nc.scalar.activation(th[:], Cst[:], ACT.Tanh)
                    hf32 = tmpp.tile([128, 16], mybir.dt.float32, tag="hf32")
                    nc.vector.tensor_mul(hf32[:], S[:, 32:48], th[:])
                    nc.scalar.copy(
                        Hr[:, :, dst:dst + BL],
                        hf32[:].rearrange("p (k b) -> p k b", k=2),
                    )

            # ---- emissions ----
            EM = wp.tile([K, ROWS], mybir.dt.float32, tag="EM")
            for c in range(NCH):
                pe = psE.tile([K, 512], mybir.dt.float32)
                rhs = [
                    HFr[:, 0, BL + 512 * c:BL + 512 * (c + 1)],
                    HFr[:, 1, BL + 512 * c:BL + 512 * (c + 1)],
                    HBr[:, 0, 512 * c:512 * (c + 1)],
                    HBr[:, 1, 512 * c:512 * (c + 1)],
                ]
                for ki in range(4):
                    nc.tensor.matmul(pe[:], wtag[ki][:], rhs[ki],
                                     start=(ki == 0), stop=(ki == 3))
                nc.vector.tensor_copy(EM[:, 512 * c:512 * (c + 1)], pe[:])
            nc.gpsimd.dma_start(emT[:, :], EM[:])
    nc.compile()
    return nc


def _sigmoid(x):
    return 1.0 / (1.0 + np.exp(-x))


def _lstm_dir_from_xg(xg, Whh):
    Bs, Ts, Gd = xg.shape
    Hd = Gd // 4
    WhhT = np.ascontiguousarray(Whh.T)
    h = np.zeros((Bs, Hd), F32)
    c = np.zeros((Bs, Hd), F32)
    out = np.empty((Bs, Ts, Hd), F32)
    for t in range(Ts):
        g = xg[:, t] + h @ WhhT
        i = _sigmoid(g[:, :Hd])
        f = _sigmoid(g[:, Hd:2 * Hd])
        gg = np.tanh(g[:, 2 * Hd:3 * Hd])
        o = _sigmoid(g[:, 3 * Hd:])
        c = f * c + i * gg
        h = o * np.tanh(c)
        out[:, t] = h
    return out


def _lstm_dir_host(x, Wih, Whh, b):
    xg = np.einsum('bti,gi->btg', x, Wih, optimize=True) + b
    return _lstm_dir_from_xg(xg.astype(F32), Whh)


def _logsumexp(a, axis):
    m = np.max(a, axis=axis, keepdims=True)
    return (m + np.log(np.sum(np.exp(a - m), axis=axis,
                              keepdims=True))).squeeze(axis)


def _emissions_host(x, wWih_f, wWhh_f, wb_f, wWih_b, wWhh_b, wb_b, Wtag):
    """fp32 fallback if the device path fails."""
    hf = _lstm_dir_host(x, wWih_f, wWhh_f, wb_f)
    hb = _lstm_dir_host(x[:, ::-1], wWih_b, wWhh_b, wb_b)[:, ::-1]
    seq = np.concatenate([hf, hb], axis=2)
    return np.einsum('bth,kh->btk', seq, Wtag, optimize=True)


def _emissions_device(x, wWih_f, wWhh_f, wb_f, wWih_b, wWhh_b, wb_b, Wtag):
    if "nc" not in _CACHE:
        _CACHE["nc"] = _build_nc()
    nc = _CACHE["nc"]
    wihT = np.ascontiguousarray(np.concatenate([
        np.concatenate([wWih_f, wb_f[:, None]], 1)[PERM].T,
        np.concatenate([wWih_b, wb_b[:, None]], 1)[PERM].T,
    ], axis=1)).astype(BF16)
    whhT = np.ascontiguousarray(
        np.concatenate([wWhh_f[PERM].T, wWhh_b[PERM].T], 1)).astype(BF16)
    wtagT = np.ascontiguousarray(Wtag.T).astype(BF16)
    in_maps = []
    for ci in range(N_CORES):
        xs = x[ci * BL:(ci + 1) * BL]                      # (BL, T, 320)
        xa = np.concatenate([xs, np.ones((BL, T, 1), F32)], 2)
        xTc = np.ascontiguousarray(
            xa.transpose(1, 0, 2).reshape(ROWS, KIN).T).astype(BF16)
        in_maps.append({"xT": xTc, "wihT": wihT, "whhT": whhT,
                        "wtagT": wtagT})
    _CACHE["last_in_maps"] = in_maps
    # First exec on a freshly-compiled NEFF occasionally hits a transient
    # NRT_EXEC_UNIT_UNRECOVERABLE on this axon tunnel; retry (with a fresh
    # build on the second failure).
    res = None
    for attempt in range(3):
        try:
            res = run_bass_kernel_spmd(nc, in_maps,
                                       core_ids=list(range(N_CORES)))
            break
        except Exception:
            if attempt == 2:
                raise
            import time as _time
            _time.sleep(5)
            if attempt == 1:
                _CACHE.pop("nc", None)
                nc = _CACHE.setdefault("nc", _build_nc())
    em = np.empty((B, T, K), F32)
    for ci in range(N_CORES):
        emc = res.results[ci]["emT"]                       # (K, ROWS)
        em[ci * BL:(ci + 1) * BL] = emc.T.reshape(T, BL, K).transpose(1, 0, 2)
    return em


def kernel(char_tensor, token_tensor, tags, mask, emb,
           cWih_f, cWhh_f, cb_f, cWih_b, cWhh_b, cb_b,
           wWih_f, wWhh_f, wb_f, wWih_b, wWhh_b, wb_b,
           Wtag, btag, start_t, end_t, trans):
    f32 = lambda a: np.asarray(a, F32)
    char_tensor = f32(char_tensor)
    emb = f32(emb)
    token_tensor = np.asarray(token_tensor).astype(np.int64)
    tags_i = np.asarray(tags).astype(np.int64)
    mask_b = np.asarray(mask).astype(bool)

    # --- char BiLSTM (tiny) + embedding gather on host ---
    cf = _lstm_dir_host(char_tensor, f32(cWih_f), f32(cWhh_f), f32(cb_f))
    cb = _lstm_dir_host(char_tensor[:, ::-1], f32(cWih_b), f32(cWhh_b),
                        f32(cb_b))[:, ::-1]
    word_emb = emb[token_tensor]                                  # (B,T,300)
    x = np.concatenate([cf, cb, word_emb], axis=2)                # (B,T,320)

    # --- word BiLSTM + emissions on the 8 NeuronCores ---
    args = (x, f32(wWih_f), f32(wWhh_f), f32(wb_f), f32(wWih_b),
            f32(wWhh_b), f32(wb_b), f32(Wtag))
    try:
        em = _emissions_device(*args)
    except Exception:
        em = _emissions_host(*args)
    em = em + f32(btag)

    # --- CRF NLL on host ---
    em = np.swapaxes(em, 0, 1)                                    # (T,B,K)
    tg = np.swapaxes(tags_i, 0, 1)
    m = np.swapaxes(mask_b, 0, 1).astype(F32)
    start_t, end_t, trans = f32(start_t), f32(end_t), f32(trans)
    bidx = np.arange(B)
    e_sc = np.take_along_axis(em, tg[..., None], axis=-1)[..., 0]  # (T,B)
    num = start_t[tg[0]] + e_sc[0]
    num = num + np.sum((trans[tg[:-1], tg[1:]] + e_sc[1:]) * m[1:], axis=0)
    last = (np.sum(m, axis=0) - 1).astype(np.int64)
    num = num + end_t[tg[last, bidx]]
    alpha = start_t[None, :] + em[0]
    for t in range(1, T):
        nxt = _logsumexp(alpha[:, :, None] + trans[None, :, :]
                         + em[t][:, None, :], axis=1)
        alpha = np.where(m[t][:, None] > 0, nxt, alpha)
    den = _logsumexp(alpha + end_t[None, :], axis=1)
    return F32(-np.sum(num - den))


# revision 4
# speedup vs baseline: 8.6914x; 4.4337x over previous
"""BiLSTM-CRF loss for nn_BiLSTM_CRF_68152541053203 on 8 TRN2 NeuronCores.

Sharding: data-parallel over batch (B=64 -> BL=8 rows/core). Each core runs
the full word-BiLSTM on its batch shard entirely on-device:
  - input projections xgT = Wih_aug @ x_augT (LSTM bias folded via an
    appended ones-column on x),
  - the 256-step forward+backward LSTM recurrences,
  - emissions emT = Wtag @ seqT.
Device layout keeps hidden/gate dims on SBUF partitions and batch on the
free dim, with bf16 matmul operands (fp32 cell state), so the sequential
recurrence is TensorE-bound instead of ACT/DVE-bound. Only the (20, 2048)
emission logits per core return to the host (~1.3 MB total instead of the
134 MB of gate pre-activations a host-recurrence design needs), where the
cheap char-BiLSTM, embedding gather and CRF run in numpy fp32.
"""

import numpy as np
import ml_dtypes

import concourse.bacc as bacc
import concourse.mybir as mybir
import concourse.tile as tile
from concourse.bass_utils import run_bass_kernel_spmd

BF16 = ml_dtypes.bfloat16
F32 = np.float32

N_CORES = 8
B, T = 64, 256
CIN, CH = 25, 10
EMB_IN, H = 320, 256
K = 20
BL = B // N_CORES          # 8 batch rows per core
ROWS = BL * T              # 2048
KIN = EMB_IN + 1           # ones column folds the LSTM input bias
G = 4 * H                  # 1024 gates per direction
KT = [(0, 128), (128, 128), (256, KIN - 256)]
HC = BL * (T + 1)          # h columns per k-section incl. zero pad
ACT = mybir.ActivationFunctionType

# PyTorch gate rows [i, f, g, o] -> device tile order [i, f, o, g]
PERM = np.r_[0:H, H:2 * H, 3 * H:4 * H, 2 * H:3 * H]

_CACHE = {}


def _build_nc():
    nc = bacc.Bacc("TRN2", target_bir_lowering=False, debug=False,
                   num_devices=N_CORES)
    xT = nc.dram_tensor("xT", [KIN, ROWS], mybir.dt.bfloat16,
                        kind="ExternalInput").ap()
    wihT = nc.dram_tensor("wihT", [KIN, 2 * G], mybir.dt.bfloat16,
                          kind="ExternalInput").ap()
    whhT = nc.dram_tensor("whhT", [H, 2 * G], mybir.dt.bfloat16,
                          kind="ExternalInput").ap()
    wtagT = nc.dram_tensor("wtagT", [2 * H, K], mybir.dt.bfloat16,
                           kind="ExternalInput").ap()
    emT = nc.dram_tensor("emT", [K, ROWS], mybir.dt.float32,
                         kind="ExternalOutput").ap()

    with tile.TileContext(nc) as tc:
        with (
            tc.tile_pool(name="w", bufs=1) as wp,
            tc.tile_pool(name="psX", bufs=3, space="PSUM") as psX,
            tc.tile_pool(name="psR", bufs=3, space="PSUM") as psR,
            tc.tile_pool(name="psE", bufs=2, space="PSUM") as psE,
            tc.tile_pool(name="gact", bufs=4) as gact,
            tc.tile_pool(name="tmp", bufs=6) as tmpp,
        ):
            xk, wih = [], []
            for i, (k0, kn) in enumerate(KT):
                t = wp.tile([kn, ROWS], mybir.dt.bfloat16, tag=f"xk{i}")
                nc.gpsimd.dma_start(t[:], xT[k0:k0 + kn, :])
                xk.append(t)
                t = wp.tile([kn, 2 * G], mybir.dt.bfloat16, tag=f"wih{i}")
                nc.gpsimd.dma_start(t[:], wihT[k0:k0 + kn, :])
                wih.append(t)
            whh = []
            for i in range(2):
                t = wp.tile([128, 2 * G], mybir.dt.bfloat16, tag=f"whh{i}")
                nc.gpsimd.dma_start(t[:], whhT[128 * i:128 * (i + 1), :])
                whh.append(t)
            wtag = []
            for i in range(4):
                t = wp.tile([128, K], mybir.dt.bfloat16, tag=f"wtag{i}")
                nc.gpsimd.dma_start(t[:], wtagT[128 * i:128 * (i + 1), :])
                wtag.append(t)

            # xgT mega-tile: free layout (t, gate-tile m 0..15, batch b);
            # m 0..7 = fwd tiles [i0 i1 f0 f1 o0 o1 g0 g1], m 8..15 = bwd.
            XG = wp.tile([128, T * 128], mybir.dt.bfloat16, tag="XG")
            XGr = XG[:].rearrange("p (t m b) -> p t m b", t=T, m=16, b=BL)
            # h sequences, bf16, one tile per dir; free layout (k-section, col)
            # fwd: h_t at col 8*(t+1) (zeros at 0:8); bwd: h_t at col 8*t
            # (zeros at 8T:8(T+1)).
            HF = wp.tile([128, 2 * HC], mybir.dt.bfloat16, tag="HF")
            HB = wp.tile([128, 2 * HC], mybir.dt.bfloat16, tag="HB")
            HFr = HF[:].rearrange("p (k c) -> p k c", k=2)
            HBr = HB[:].rearrange("p (k c) -> p k c", k=2)
            nc.vector.memset(HFr[:, :, 0:BL], 0.0)
            nc.vector.memset(HBr[:, :, BL * T:BL * (T + 1)], 0.0)
            CF = wp.tile([128, 16], mybir.dt.float32, tag="CF")
            CB = wp.tile([128, 16], mybir.dt.float32, tag="CB")
            nc.vector.memset(CF[:], 0.0)
            nc.vector.memset(CB[:], 0.0)

            # ---- input projections ----
            NCH = ROWS // 512
            chunk_order = [0, NCH - 1] + list(range(1, NCH - 1))
            for m in range(16):
                for c in chunk_order:
                    ps = psX.tile([128, 512], mybir.dt.float32)
                    for ki, (k0, kn) in enumerate(KT):
                        nc.tensor.matmul(
                            ps[:],
                            wih[ki][:, 128 * m:128 * (m + 1)],
                            xk[ki][:, 512 * c:512 * (c + 1)],
                            start=(ki == 0), stop=(ki == len(KT) - 1),
                        )
                    tpc = 512 // BL
                    nc.vector.tensor_copy(
                        XGr[:, tpc * c:tpc * (c + 1), m, :],
                        ps[:].rearrange("p (t b) -> p t b", b=BL),
                    )

            # ---- recurrence (fwd and bwd as independent chains) ----
            for t in range(T):
                for d in range(2):
                    td = t if d == 0 else T - 1 - t
                    Hr = HFr if d == 0 else HBr
                    Cst = CF if d == 0 else CB
                    src = BL * td if d == 0 else BL * (td + 1)
                    dst = BL * (td + 1) if d == 0 else BL * td
                    ps = psR.tile([128, 64], mybir.dt.float32)
                    for g in range(8):
                        for k in range(2):
                            nc.tensor.matmul(
                                ps[:, 8 * g:8 * (g + 1)],
                                whh[k][:, G * d + 128 * g:
                                       G * d + 128 * (g + 1)],
                                Hr[:, k, src:src + BL],
                                start=(k == 0), stop=(k == 1),
                            )
                    S = gact.tile([128, 64], mybir.dt.float32)
                    nc.vector.tensor_add(
                        S[:].rearrange("p (m b) -> p m b", b=BL),
                        ps[:].rearrange("p (m b) -> p m b", b=BL),
                        XGr[:, td, 8 * d:8 * (d + 1), :],
                    )
                    nc.scalar.activation(S[:, 0:48], S[:, 0:48], ACT.Sigmoid)
                    nc.scalar.activation(S[:, 48:64], S[:, 48:64], ACT.Tanh)
                    t1 = tmpp.tile([128, 16], mybir.dt.float32, tag="t1")
                    t2 = tmpp.tile([128, 16], mybir.dt.float32, tag="t2")
                    nc.vector.tensor_mul(t1[:], S[:, 0:16], S[:, 48:64])
                    nc.vector.tensor_mul(t2[:], S[:, 16:32], Cst[:])
                    nc.vector.tensor_add(Cst[:], t1[:], t2[:])
                    th = tmpp.tile([128, 16], mybir.dt.float32, tag="th")
                    nc.scalar.activation(th[:], Cst[:], ACT.Tanh)
                    hf32 = tmpp.tile([128, 16], mybir.dt.float32, tag="hf32")
                    nc.vector.tensor_mul(hf32[:], S[:, 32:48], th[:])
                    nc.scalar.copy(
                        Hr[:, :, dst:dst + BL],
                        hf32[:].rearrange("p (k b) -> p k b", k=2),
                    )

            # ---- emissions ----
            EM = wp.tile([K, ROWS], mybir.dt.float32, tag="EM")
            for c in range(NCH):
                pe = psE.tile([K, 512], mybir.dt.float32)
                rhs = [
                    HFr[:, 0, BL + 512 * c:BL + 512 * (c + 1)],
                    HFr[:, 1, BL + 512 * c:BL + 512 * (c + 1)],
                    HBr[:, 0, 512 * c:512 * (c + 1)],
                    HBr[:, 1, 512 * c:512 * (c + 1)],
                ]
                for ki in range(4):
                    nc.tensor.matmul(pe[:], wtag[ki][:], rhs[ki],
                                     start=(ki == 0), stop=(ki == 3))
                nc.vector.tensor_copy(EM[:, 512 * c:512 * (c + 1)], pe[:])
            nc.gpsimd.dma_start(emT[:, :], EM[:])
    nc.compile()
    return nc


def _make_runner(nc):
    """Build a persistent jitted SPMD dispatcher for `nc`.

    run_bass_kernel_spmd re-creates (and re-jits) its shard_map closure on
    every call, costing ~1-2 s of XLA retrace/compile per invocation. This
    builds the same program once and caches the jitted callable, so repeat
    calls pay only host->device transfer + execution.
    """
    import jax
    from jax.sharding import Mesh, PartitionSpec
    from jax.experimental.shard_map import shard_map
    from concourse import bass2jax

    bass2jax.install_neuronx_cc_hook()
    partition_name = (nc.partition_id_tensor.name
                      if nc.partition_id_tensor else None)
    in_names, out_names, out_avals, zero_shapes = [], [], [], []
    for alloc in nc.m.functions[0].allocations:
        if not isinstance(alloc, mybir.MemoryLocationSet):
            continue
        name = alloc.memorylocations[0].name
        if alloc.kind == "ExternalInput":
            if name != partition_name:
                in_names.append(name)
        elif alloc.kind == "ExternalOutput":
            shape = tuple(alloc.tensor_shape)
            dtype = mybir.dt.np(alloc.dtype)
            out_names.append(name)
            out_avals.append(jax.core.ShapedArray(shape, dtype))
            zero_shapes.append((shape, dtype))
    n_params = len(in_names)
    all_names = list(in_names) + list(out_names)
    if partition_name is not None:
        all_names.append(partition_name)
    donate = tuple(range(n_params, n_params + len(out_names)))

    def _body(*args):
        operands = list(args)
        if partition_name is not None:
            operands.append(bass2jax.partition_id_tensor())
        outs = bass2jax._bass_exec_p.bind(
            *operands,
            out_avals=tuple(out_avals),
            in_names=tuple(all_names),
            out_names=tuple(out_names),
            lowering_input_output_aliases=(),
            sim_require_finite=True,
            sim_require_nnan=True,
            nc=nc,
        )
        return tuple(outs)

    devices = jax.devices()[:N_CORES]
    mesh = Mesh(np.asarray(devices), ("core",))
    specs = (PartitionSpec("core"),) * (n_params + len(out_names))
    sharded = jax.jit(
        shard_map(_body, mesh=mesh, in_specs=specs,
                  out_specs=(PartitionSpec("core"),) * len(out_names),
                  check_rep=False),
        donate_argnums=donate, keep_unused=True,
    )

    def run(in_maps):
        concat_in = [
            np.concatenate([np.asarray(in_maps[c][n]) for c in range(N_CORES)],
                           axis=0)
            for n in in_names
        ]
        concat_zeros = [np.zeros((N_CORES * s[0], *s[1:]), d)
                        for s, d in zero_shapes]
        out_arrs = sharded(*concat_in, *concat_zeros)
        return [
            {n: np.asarray(out_arrs[i]).reshape(N_CORES, *zero_shapes[i][0])[c]
             for i, n in enumerate(out_names)}
            for c in range(N_CORES)
        ]
    return run


def _run_spmd(nc, in_maps):
    if _CACHE.get("runner_nc") is not nc:
        _CACHE["runner"] = _make_runner(nc)
        _CACHE["runner_nc"] = nc
    return _CACHE["runner"](in_maps)


def _sigmoid(x):
    return 1.0 / (1.0 + np.exp(-x))


def _lstm_dir_from_xg(xg, Whh):
    Bs, Ts, Gd = xg.shape
    Hd = Gd // 4
    WhhT = np.ascontiguousarray(Whh.T)
    h = np.zeros((Bs, Hd), F32)
    c = np.zeros((Bs, Hd), F32)
    out = np.empty((Bs, Ts, Hd), F32)
    for t in range(Ts):
        g = xg[:, t] + h @ WhhT
        i = _sigmoid(g[:, :Hd])
        f = _sigmoid(g[:, Hd:2 * Hd])
        gg = np.tanh(g[:, 2 * Hd:3 * Hd])
        o = _sigmoid(g[:, 3 * Hd:])
        c = f * c + i * gg
        h = o * np.tanh(c)
        out[:, t] = h
    return out


def _lstm_dir_host(x, Wih, Whh, b):
    xg = np.einsum('bti,gi->btg', x, Wih, optimize=True) + b
    return _lstm_dir_from_xg(xg.astype(F32), Whh)


def _logsumexp(a, axis):
    m = np.max(a, axis=axis, keepdims=True)
    return (m + np.log(np.sum(np.exp(a - m), axis=axis,
                              keepdims=True))).squeeze(axis)


def _emissions_host(x, wWih_f, wWhh_f, wb_f, wWih_b, wWhh_b, wb_b, Wtag):
    """fp32 fallback if the device path fails."""
    hf = _lstm_dir_host(x, wWih_f, wWhh_f, wb_f)
    hb = _lstm_dir_host(x[:, ::-1], wWih_b, wWhh_b, wb_b)[:, ::-1]
    seq = np.concatenate([hf, hb], axis=2)
    return np.einsum('bth,kh->btk', seq, Wtag, optimize=True)


def _emissions_device(x, wWih_f, wWhh_f, wb_f, wWih_b, wWhh_b, wb_b, Wtag):
    if "nc" not in _CACHE:
        _CACHE["nc"] = _build_nc()
    nc = _CACHE["nc"]
    wihT = np.ascontiguousarray(np.concatenate([
        np.concatenate([wWih_f, wb_f[:, None]], 1)[PERM].T,
        np.concatenate([wWih_b, wb_b[:, None]], 1)[PERM].T,
    ], axis=1)).astype(BF16)
    whhT = np.ascontiguousarray(
        np.concatenate([wWhh_f[PERM].T, wWhh_b[PERM].T], 1)).astype(BF16)
    wtagT = np.ascontiguousarray(Wtag.T).astype(BF16)
    in_maps = []
    for ci in range(N_CORES):
        xs = x[ci * BL:(ci + 1) * BL]                      # (BL, T, 320)
        xa = np.concatenate([xs, np.ones((BL, T, 1), F32)], 2)
        xTc = np.ascontiguousarray(
            xa.transpose(1, 0, 2).reshape(ROWS, KIN).T).astype(BF16)
        in_maps.append({"xT": xTc, "wihT": wihT, "whhT": whhT,
                        "wtagT": wtagT})
    _CACHE["last_in_maps"] = in_maps
    # First exec on a freshly-compiled NEFF occasionally hits a transient
    # NRT_EXEC_UNIT_UNRECOVERABLE on this axon tunnel; retry (with a fresh
    # build on the second failure).
    results = None
    for attempt in range(3):
        try:
            results = _run_spmd(nc, in_maps)
            break
        except Exception:
            if attempt == 2:
                raise
            import time as _time
            _time.sleep(5)
            if attempt == 1:
                _CACHE.pop("nc", None)
                _CACHE.pop("runner_nc", None)
                nc = _CACHE.setdefault("nc", _build_nc())
    em = np.empty((B, T, K), F32)
    for ci in range(N_CORES):
        emc = results[ci]["emT"]                           # (K, ROWS)
        em[ci * BL:(ci + 1) * BL] = emc.T.reshape(T, BL, K).transpose(1, 0, 2)
    return em


def kernel(char_tensor, token_tensor, tags, mask, emb,
           cWih_f, cWhh_f, cb_f, cWih_b, cWhh_b, cb_b,
           wWih_f, wWhh_f, wb_f, wWih_b, wWhh_b, wb_b,
           Wtag, btag, start_t, end_t, trans):
    f32 = lambda a: np.asarray(a, F32)
    char_tensor = f32(char_tensor)
    emb = f32(emb)
    token_tensor = np.asarray(token_tensor).astype(np.int64)
    tags_i = np.asarray(tags).astype(np.int64)
    mask_b = np.asarray(mask).astype(bool)

    # --- char BiLSTM (tiny) + embedding gather on host ---
    cf = _lstm_dir_host(char_tensor, f32(cWih_f), f32(cWhh_f), f32(cb_f))
    cb = _lstm_dir_host(char_tensor[:, ::-1], f32(cWih_b), f32(cWhh_b),
                        f32(cb_b))[:, ::-1]
    word_emb = emb[token_tensor]                                  # (B,T,300)
    x = np.concatenate([cf, cb, word_emb], axis=2)                # (B,T,320)

    # --- word BiLSTM + emissions on the 8 NeuronCores ---
    args = (x, f32(wWih_f), f32(wWhh_f), f32(wb_f), f32(wWih_b),
            f32(wWhh_b), f32(wb_b), f32(Wtag))
    try:
        em = _emissions_device(*args)
    except Exception:
        em = _emissions_host(*args)
    em = em + f32(btag)

    # --- CRF NLL on host ---
    em = np.swapaxes(em, 0, 1)                                    # (T,B,K)
    tg = np.swapaxes(tags_i, 0, 1)
    m = np.swapaxes(mask_b, 0, 1).astype(F32)
    start_t, end_t, trans = f32(start_t), f32(end_t), f32(trans)
    bidx = np.arange(B)
    e_sc = np.take_along_axis(em, tg[..., None], axis=-1)[..., 0]  # (T,B)
    num = start_t[tg[0]] + e_sc[0]
    num = num + np.sum((trans[tg[:-1], tg[1:]] + e_sc[1:]) * m[1:], axis=0)
    last = (np.sum(m, axis=0) - 1).astype(np.int64)
    num = num + end_t[tg[last, bidx]]
    alpha = start_t[None, :] + em[0]
    for t in range(1, T):
        nxt = _logsumexp(alpha[:, :, None] + trans[None, :, :]
                         + em[t][:, None, :], axis=1)
        alpha = np.where(m[t][:, None] > 0, nxt, alpha)
    den = _logsumexp(alpha + end_t[None, :], axis=1)
    return F32(-np.sum(num - den))


# revision 5
# speedup vs baseline: 9.1098x; 1.0481x over previous
"""BiLSTM-CRF loss for nn_BiLSTM_CRF_68152541053203 on 8 TRN2 NeuronCores.

Sharding: data-parallel over batch (B=64 -> BL=8 rows/core). Each core runs
the full word-BiLSTM on its batch shard entirely on-device:
  - input projections xgT = Wih_aug @ x_augT (LSTM bias folded via an
    appended ones-column on x),
  - the 256-step forward+backward LSTM recurrences,
  - emissions emT = Wtag @ seqT.
Device layout keeps hidden/gate dims on SBUF partitions and batch on the
free dim, with bf16 matmul operands (fp32 cell state), so the sequential
recurrence is TensorE-bound instead of ACT/DVE-bound. Only the (20, 2048)
emission logits per core return to the host (~1.3 MB total instead of the
134 MB of gate pre-activations a host-recurrence design needs), where the
cheap char-BiLSTM, embedding gather and CRF run in numpy fp32.
"""

import numpy as np
import ml_dtypes

import concourse.bacc as bacc
import concourse.mybir as mybir
import concourse.tile as tile
from concourse.bass_utils import run_bass_kernel_spmd

BF16 = ml_dtypes.bfloat16
F32 = np.float32

N_CORES = 8
B, T = 64, 256
CIN, CH = 25, 10
EMB_IN, H = 320, 256
K = 20
BL = B // N_CORES          # 8 batch rows per core
ROWS = BL * T              # 2048
KIN = EMB_IN + 1           # ones column folds the LSTM input bias
G = 4 * H                  # 1024 gates per direction
KT = [(0, 128), (128, 128), (256, KIN - 256)]
HC = BL * (T + 1)          # h columns per k-section incl. zero pad
ACT = mybir.ActivationFunctionType

# PyTorch gate rows [i, f, g, o] -> device tile order [i, f, o, g]
PERM = np.r_[0:H, H:2 * H, 3 * H:4 * H, 2 * H:3 * H]

_CACHE = {}


def _build_nc():
    nc = bacc.Bacc("TRN2", target_bir_lowering=False, debug=False,
                   num_devices=N_CORES)
    xT = nc.dram_tensor("xT", [KIN, ROWS], mybir.dt.bfloat16,
                        kind="ExternalInput").ap()
    wihT = nc.dram_tensor("wihT", [KIN, 2 * G], mybir.dt.bfloat16,
                          kind="ExternalInput").ap()
    whhT = nc.dram_tensor("whhT", [H, 2 * G], mybir.dt.bfloat16,
                          kind="ExternalInput").ap()
    wtagT = nc.dram_tensor("wtagT", [2 * H, K], mybir.dt.bfloat16,
                           kind="ExternalInput").ap()
    emT = nc.dram_tensor("emT", [K, ROWS], mybir.dt.float32,
                         kind="ExternalOutput").ap()

    with tile.TileContext(nc) as tc:
        with (
            tc.tile_pool(name="w", bufs=1) as wp,
            tc.tile_pool(name="psX", bufs=3, space="PSUM") as psX,
            tc.tile_pool(name="psR", bufs=3, space="PSUM") as psR,
            tc.tile_pool(name="psE", bufs=2, space="PSUM") as psE,
            tc.tile_pool(name="gact", bufs=4) as gact,
            tc.tile_pool(name="tmp", bufs=6) as tmpp,
        ):
            xk, wih = [], []
            for i, (k0, kn) in enumerate(KT):
                t = wp.tile([kn, ROWS], mybir.dt.bfloat16, tag=f"xk{i}")
                nc.gpsimd.dma_start(t[:], xT[k0:k0 + kn, :])
                xk.append(t)
                t = wp.tile([kn, 2 * G], mybir.dt.bfloat16, tag=f"wih{i}")
                nc.gpsimd.dma_start(t[:], wihT[k0:k0 + kn, :])
                wih.append(t)
            whh = []
            for i in range(2):
                t = wp.tile([128, 2 * G], mybir.dt.bfloat16, tag=f"whh{i}")
                nc.gpsimd.dma_start(t[:], whhT[128 * i:128 * (i + 1), :])
                whh.append(t)
            wtag = []
            for i in range(4):
                t = wp.tile([128, K], mybir.dt.bfloat16, tag=f"wtag{i}")
                nc.gpsimd.dma_start(t[:], wtagT[128 * i:128 * (i + 1), :])
                wtag.append(t)

            # xgT mega-tile: free layout (t, gate-tile m 0..15, batch b);
            # m 0..7 = fwd tiles [i0 i1 f0 f1 o0 o1 g0 g1], m 8..15 = bwd.
            XG = wp.tile([128, T * 128], mybir.dt.bfloat16, tag="XG")
            XGr = XG[:].rearrange("p (t m b) -> p t m b", t=T, m=16, b=BL)
            # h sequences, bf16, one tile per dir; free layout (k-section, col)
            # fwd: h_t at col 8*(t+1) (zeros at 0:8); bwd: h_t at col 8*t
            # (zeros at 8T:8(T+1)).
            HF = wp.tile([128, 2 * HC], mybir.dt.bfloat16, tag="HF")
            HB = wp.tile([128, 2 * HC], mybir.dt.bfloat16, tag="HB")
            HFr = HF[:].rearrange("p (k c) -> p k c", k=2)
            HBr = HB[:].rearrange("p (k c) -> p k c", k=2)
            nc.vector.memset(HFr[:, :, 0:BL], 0.0)
            nc.vector.memset(HBr[:, :, BL * T:BL * (T + 1)], 0.0)
            CF = wp.tile([128, 16], mybir.dt.float32, tag="CF")
            CB = wp.tile([128, 16], mybir.dt.float32, tag="CB")
            nc.vector.memset(CF[:], 0.0)
            nc.vector.memset(CB[:], 0.0)

            # ---- input projections ----
            NCH = ROWS // 512
            chunk_order = [0, NCH - 1] + list(range(1, NCH - 1))
            for m in range(16):
                for c in chunk_order:
                    ps = psX.tile([128, 512], mybir.dt.float32)
                    for ki, (k0, kn) in enumerate(KT):
                        nc.tensor.matmul(
                            ps[:],
                            wih[ki][:, 128 * m:128 * (m + 1)],
                            xk[ki][:, 512 * c:512 * (c + 1)],
                            start=(ki == 0), stop=(ki == len(KT) - 1),
                        )
                    tpc = 512 // BL
                    nc.vector.tensor_copy(
                        XGr[:, tpc * c:tpc * (c + 1), m, :],
                        ps[:].rearrange("p (t b) -> p t b", b=BL),
                    )

            # ---- recurrence (fwd and bwd as independent chains) ----
            for t in range(T):
                for d in range(2):
                    td = t if d == 0 else T - 1 - t
                    Hr = HFr if d == 0 else HBr
                    Cst = CF if d == 0 else CB
                    src = BL * td if d == 0 else BL * (td + 1)
                    dst = BL * (td + 1) if d == 0 else BL * td
                    ps = psR.tile([128, 64], mybir.dt.float32)
                    for g in range(8):
                        for k in range(2):
                            nc.tensor.matmul(
                                ps[:, 8 * g:8 * (g + 1)],
                                whh[k][:, G * d + 128 * g:
                                       G * d + 128 * (g + 1)],
                                Hr[:, k, src:src + BL],
                                start=(k == 0), stop=(k == 1),
                            )
                    S = gact.tile([128, 64], mybir.dt.float32)
                    nc.vector.tensor_add(
                        S[:].rearrange("p (m b) -> p m b", b=BL),
                        ps[:].rearrange("p (m b) -> p m b", b=BL),
                        XGr[:, td, 8 * d:8 * (d + 1), :],
                    )
                    nc.scalar.activation(S[:, 0:48], S[:, 0:48], ACT.Sigmoid)
                    nc.scalar.activation(S[:, 48:64], S[:, 48:64], ACT.Tanh)
                    t1 = tmpp.tile([128, 16], mybir.dt.float32, tag="t1")
                    t2 = tmpp.tile([128, 16], mybir.dt.float32, tag="t2")
                    nc.vector.tensor_mul(t1[:], S[:, 0:16], S[:, 48:64])
                    nc.vector.tensor_mul(t2[:], S[:, 16:32], Cst[:])
                    nc.vector.tensor_add(Cst[:], t1[:], t2[:])
                    th = tmpp.tile([128, 16], mybir.dt.float32, tag="th")
                    nc.scalar.activation(th[:], Cst[:], ACT.Tanh)
                    hf32 = tmpp.tile([128, 16], mybir.dt.float32, tag="hf32")
                    nc.vector.tensor_mul(hf32[:], S[:, 32:48], th[:])
                    nc.scalar.copy(
                        Hr[:, :, dst:dst + BL],
                        hf32[:].rearrange("p (k b) -> p k b", k=2),
                    )

            # ---- emissions ----
            EM = wp.tile([K, ROWS], mybir.dt.float32, tag="EM")
            for c in range(NCH):
                pe = psE.tile([K, 512], mybir.dt.float32)
                rhs = [
                    HFr[:, 0, BL + 512 * c:BL + 512 * (c + 1)],
                    HFr[:, 1, BL + 512 * c:BL + 512 * (c + 1)],
                    HBr[:, 0, 512 * c:512 * (c + 1)],
                    HBr[:, 1, 512 * c:512 * (c + 1)],
                ]
                for ki in range(4):
                    nc.tensor.matmul(pe[:], wtag[ki][:], rhs[ki],
                                     start=(ki == 0), stop=(ki == 3))
                nc.vector.tensor_copy(EM[:, 512 * c:512 * (c + 1)], pe[:])
            nc.gpsimd.dma_start(emT[:, :], EM[:])
    nc.compile()
    return nc


def _make_runner(nc):
    """Build a persistent jitted SPMD dispatcher for `nc`.

    run_bass_kernel_spmd re-creates (and re-jits) its shard_map closure on
    every call, costing ~1-2 s of XLA retrace/compile per invocation. This
    builds the same program once and caches the jitted callable, so repeat
    calls pay only host->device transfer + execution.
    """
    import jax
    from jax.sharding import Mesh, PartitionSpec
    from jax.experimental.shard_map import shard_map
    from concourse import bass2jax

    bass2jax.install_neuronx_cc_hook()
    partition_name = (nc.partition_id_tensor.name
                      if nc.partition_id_tensor else None)
    in_names, out_names, out_avals, zero_shapes = [], [], [], []
    for alloc in nc.m.functions[0].allocations:
        if not isinstance(alloc, mybir.MemoryLocationSet):
            continue
        name = alloc.memorylocations[0].name
        if alloc.kind == "ExternalInput":
            if name != partition_name:
                in_names.append(name)
        elif alloc.kind == "ExternalOutput":
            shape = tuple(alloc.tensor_shape)
            dtype = mybir.dt.np(alloc.dtype)
            out_names.append(name)
            out_avals.append(jax.core.ShapedArray(shape, dtype))
            zero_shapes.append((shape, dtype))
    n_params = len(in_names)
    all_names = list(in_names) + list(out_names)
    if partition_name is not None:
        all_names.append(partition_name)
    donate = tuple(range(n_params, n_params + len(out_names)))

    def _body(*args):
        operands = list(args)
        if partition_name is not None:
            operands.append(bass2jax.partition_id_tensor())
        outs = bass2jax._bass_exec_p.bind(
            *operands,
            out_avals=tuple(out_avals),
            in_names=tuple(all_names),
            out_names=tuple(out_names),
            lowering_input_output_aliases=(),
            sim_require_finite=True,
            sim_require_nnan=True,
            nc=nc,
        )
        return tuple(outs)

    devices = jax.devices()[:N_CORES]
    mesh = Mesh(np.asarray(devices), ("core",))
    # Inputs identical across cores (weights) are passed replicated — one
    # host->device copy instead of 8 concatenated shards over the tunnel.
    replicated = {"wihT", "whhT", "wtagT"}
    in_specs = tuple(PartitionSpec() if n in replicated
                     else PartitionSpec("core") for n in in_names)
    in_specs += (PartitionSpec("core"),) * len(out_names)
    sharded = jax.jit(
        shard_map(_body, mesh=mesh, in_specs=in_specs,
                  out_specs=(PartitionSpec("core"),) * len(out_names),
                  check_rep=False),
        donate_argnums=donate, keep_unused=True,
    )

    def run(in_maps):
        concat_in = [
            np.asarray(in_maps[0][n]) if n in replicated
            else np.concatenate([np.asarray(in_maps[c][n])
                                 for c in range(N_CORES)], axis=0)
            for n in in_names
        ]
        concat_zeros = [np.zeros((N_CORES * s[0], *s[1:]), d)
                        for s, d in zero_shapes]
        out_arrs = sharded(*concat_in, *concat_zeros)
        return [
            {n: np.asarray(out_arrs[i]).reshape(N_CORES, *zero_shapes[i][0])[c]
             for i, n in enumerate(out_names)}
            for c in range(N_CORES)
        ]
    return run


def _run_spmd(nc, in_maps):
    if _CACHE.get("runner_nc") is not nc:
        _CACHE["runner"] = _make_runner(nc)
        _CACHE["runner_nc"] = nc
    return _CACHE["runner"](in_maps)


def _sigmoid(x):
    return 1.0 / (1.0 + np.exp(-x))


def _lstm_dir_from_xg(xg, Whh):
    Bs, Ts, Gd = xg.shape
    Hd = Gd // 4
    WhhT = np.ascontiguousarray(Whh.T)
    h = np.zeros((Bs, Hd), F32)
    c = np.zeros((Bs, Hd), F32)
    out = np.empty((Bs, Ts, Hd), F32)
    for t in range(Ts):
        g = xg[:, t] + h @ WhhT
        i = _sigmoid(g[:, :Hd])
        f = _sigmoid(g[:, Hd:2 * Hd])
        gg = np.tanh(g[:, 2 * Hd:3 * Hd])
        o = _sigmoid(g[:, 3 * Hd:])
        c = f * c + i * gg
        h = o * np.tanh(c)
        out[:, t] = h
    return out


def _lstm_dir_host(x, Wih, Whh, b):
    xg = np.einsum('bti,gi->btg', x, Wih, optimize=True) + b
    return _lstm_dir_from_xg(xg.astype(F32), Whh)


def _logsumexp(a, axis):
    m = np.max(a, axis=axis, keepdims=True)
    return (m + np.log(np.sum(np.exp(a - m), axis=axis,
                              keepdims=True))).squeeze(axis)


def _emissions_host(x, wWih_f, wWhh_f, wb_f, wWih_b, wWhh_b, wb_b, Wtag):
    """fp32 fallback if the device path fails."""
    hf = _lstm_dir_host(x, wWih_f, wWhh_f, wb_f)
    hb = _lstm_dir_host(x[:, ::-1], wWih_b, wWhh_b, wb_b)[:, ::-1]
    seq = np.concatenate([hf, hb], axis=2)
    return np.einsum('bth,kh->btk', seq, Wtag, optimize=True)


def _emissions_device(x, wWih_f, wWhh_f, wb_f, wWih_b, wWhh_b, wb_b, Wtag):
    if "nc" not in _CACHE:
        _CACHE["nc"] = _build_nc()
    nc = _CACHE["nc"]
    wihT = np.ascontiguousarray(np.concatenate([
        np.concatenate([wWih_f, wb_f[:, None]], 1)[PERM].T,
        np.concatenate([wWih_b, wb_b[:, None]], 1)[PERM].T,
    ], axis=1)).astype(BF16)
    whhT = np.ascontiguousarray(
        np.concatenate([wWhh_f[PERM].T, wWhh_b[PERM].T], 1)).astype(BF16)
    wtagT = np.ascontiguousarray(Wtag.T).astype(BF16)
    in_maps = []
    for ci in range(N_CORES):
        xs = x[ci * BL:(ci + 1) * BL]                      # (BL, T, 320)
        xa = np.concatenate([xs, np.ones((BL, T, 1), F32)], 2)
        xTc = np.ascontiguousarray(
            xa.transpose(1, 0, 2).reshape(ROWS, KIN).T).astype(BF16)
        in_maps.append({"xT": xTc, "wihT": wihT, "whhT": whhT,
                        "wtagT": wtagT})
    _CACHE["last_in_maps"] = in_maps
    # First exec on a freshly-compiled NEFF occasionally hits a transient
    # NRT_EXEC_UNIT_UNRECOVERABLE on this axon tunnel; retry (with a fresh
    # build on the second failure).
    results = None
    for attempt in range(3):
        try:
            results = _run_spmd(nc, in_maps)
            break
        except Exception:
            if attempt == 2:
                raise
            import time as _time
            _time.sleep(5)
            if attempt == 1:
                _CACHE.pop("nc", None)
                _CACHE.pop("runner_nc", None)
                nc = _CACHE.setdefault("nc", _build_nc())
    em = np.empty((B, T, K), F32)
    for ci in range(N_CORES):
        emc = results[ci]["emT"]                           # (K, ROWS)
        em[ci * BL:(ci + 1) * BL] = emc.T.reshape(T, BL, K).transpose(1, 0, 2)
    return em


def kernel(char_tensor, token_tensor, tags, mask, emb,
           cWih_f, cWhh_f, cb_f, cWih_b, cWhh_b, cb_b,
           wWih_f, wWhh_f, wb_f, wWih_b, wWhh_b, wb_b,
           Wtag, btag, start_t, end_t, trans):
    f32 = lambda a: np.asarray(a, F32)
    char_tensor = f32(char_tensor)
    emb = f32(emb)
    token_tensor = np.asarray(token_tensor).astype(np.int64)
    tags_i = np.asarray(tags).astype(np.int64)
    mask_b = np.asarray(mask).astype(bool)

    # --- char BiLSTM (tiny) + embedding gather on host ---
    cf = _lstm_dir_host(char_tensor, f32(cWih_f), f32(cWhh_f), f32(cb_f))
    cb = _lstm_dir_host(char_tensor[:, ::-1], f32(cWih_b), f32(cWhh_b),
                        f32(cb_b))[:, ::-1]
    word_emb = emb[token_tensor]                                  # (B,T,300)
    x = np.concatenate([cf, cb, word_emb], axis=2)                # (B,T,320)

    # --- word BiLSTM + emissions on the 8 NeuronCores ---
    args = (x, f32(wWih_f), f32(wWhh_f), f32(wb_f), f32(wWih_b),
            f32(wWhh_b), f32(wb_b), f32(Wtag))
    try:
        em = _emissions_device(*args)
    except Exception:
        em = _emissions_host(*args)
    em = em + f32(btag)

    # --- CRF NLL on host ---
    em = np.swapaxes(em, 0, 1)                                    # (T,B,K)
    tg = np.swapaxes(tags_i, 0, 1)
    m = np.swapaxes(mask_b, 0, 1).astype(F32)
    start_t, end_t, trans = f32(start_t), f32(end_t), f32(trans)
    bidx = np.arange(B)
    e_sc = np.take_along_axis(em, tg[..., None], axis=-1)[..., 0]  # (T,B)
    num = start_t[tg[0]] + e_sc[0]
    num = num + np.sum((trans[tg[:-1], tg[1:]] + e_sc[1:]) * m[1:], axis=0)
    last = (np.sum(m, axis=0) - 1).astype(np.int64)
    num = num + end_t[tg[last, bidx]]
    alpha = start_t[None, :] + em[0]
    for t in range(1, T):
        nxt = _logsumexp(alpha[:, :, None] + trans[None, :, :]
                         + em[t][:, None, :], axis=1)
        alpha = np.where(m[t][:, None] > 0, nxt, alpha)
    den = _logsumexp(alpha + end_t[None, :], axis=1)
    return F32(-np.sum(num - den))


# revision 6
# speedup vs baseline: 17.7998x; 1.9539x over previous
"""BiLSTM-CRF loss for nn_BiLSTM_CRF_68152541053203 on 8 TRN2 NeuronCores.

Sharding: data-parallel over batch (B=64 -> BL=8 rows/core). Each core runs
the full word-BiLSTM on its batch shard entirely on-device:
  - input projections xgT = Wih_aug @ x_augT (LSTM bias folded via an
    appended ones-column on x),
  - the 256-step forward+backward LSTM recurrences,
  - emissions emT = Wtag @ seqT.
Device layout keeps hidden/gate dims on SBUF partitions and batch on the
free dim, with bf16 matmul operands (fp32 cell state), so the sequential
recurrence is TensorE-bound instead of ACT/DVE-bound. Only the (20, 2048)
emission logits per core return to the host (~1.3 MB total instead of the
134 MB of gate pre-activations a host-recurrence design needs), where the
cheap char-BiLSTM, embedding gather and CRF run in numpy fp32.
"""

import numpy as np
import ml_dtypes

import concourse.bacc as bacc
import concourse.mybir as mybir
import concourse.tile as tile
from concourse.bass_utils import run_bass_kernel_spmd

BF16 = ml_dtypes.bfloat16
F32 = np.float32

N_CORES = 8
B, T = 64, 256
CIN, CH = 25, 10
EMB_IN, H = 320, 256
K = 20
BL = B // N_CORES          # 8 batch rows per core
ROWS = BL * T              # 2048
KIN = EMB_IN + 1           # ones column folds the LSTM input bias
G = 4 * H                  # 1024 gates per direction
KT = [(0, 128), (128, 128), (256, KIN - 256)]
HC = BL * (T + 1)          # h columns per k-section incl. zero pad
ACT = mybir.ActivationFunctionType

# PyTorch gate rows [i, f, g, o] -> device tile order [i, f, o, g]
PERM = np.r_[0:H, H:2 * H, 3 * H:4 * H, 2 * H:3 * H]

_CACHE = {}


def _build_nc():
    nc = bacc.Bacc("TRN2", target_bir_lowering=False, debug=False,
                   num_devices=N_CORES)
    xT = nc.dram_tensor("xT", [KIN, ROWS], mybir.dt.bfloat16,
                        kind="ExternalInput").ap()
    wihT = nc.dram_tensor("wihT", [KIN, 2 * G], mybir.dt.bfloat16,
                          kind="ExternalInput").ap()
    whhT = nc.dram_tensor("whhT", [H, 2 * G], mybir.dt.bfloat16,
                          kind="ExternalInput").ap()
    wtagT = nc.dram_tensor("wtagT", [2 * H, K], mybir.dt.bfloat16,
                           kind="ExternalInput").ap()
    emT = nc.dram_tensor("emT", [K, ROWS], mybir.dt.float32,
                         kind="ExternalOutput").ap()

    with tile.TileContext(nc) as tc:
        with (
            tc.tile_pool(name="w", bufs=1) as wp,
            tc.tile_pool(name="psX", bufs=3, space="PSUM") as psX,
            tc.tile_pool(name="psR", bufs=3, space="PSUM") as psR,
            tc.tile_pool(name="psE", bufs=2, space="PSUM") as psE,
            tc.tile_pool(name="gact", bufs=4) as gact,
            tc.tile_pool(name="tmp", bufs=6) as tmpp,
        ):
            xk, wih = [], []
            for i, (k0, kn) in enumerate(KT):
                t = wp.tile([kn, ROWS], mybir.dt.bfloat16, tag=f"xk{i}")
                nc.gpsimd.dma_start(t[:], xT[k0:k0 + kn, :])
                xk.append(t)
                t = wp.tile([kn, 2 * G], mybir.dt.bfloat16, tag=f"wih{i}")
                nc.gpsimd.dma_start(t[:], wihT[k0:k0 + kn, :])
                wih.append(t)
            whh = []
            for i in range(2):
                t = wp.tile([128, 2 * G], mybir.dt.bfloat16, tag=f"whh{i}")
                nc.gpsimd.dma_start(t[:], whhT[128 * i:128 * (i + 1), :])
                whh.append(t)
            wtag = []
            for i in range(4):
                t = wp.tile([128, K], mybir.dt.bfloat16, tag=f"wtag{i}")
                nc.gpsimd.dma_start(t[:], wtagT[128 * i:128 * (i + 1), :])
                wtag.append(t)

            # xgT mega-tile: free layout (t, gate-tile m 0..15, batch b);
            # m 0..7 = fwd tiles [i0 i1 f0 f1 o0 o1 g0 g1], m 8..15 = bwd.
            XG = wp.tile([128, T * 128], mybir.dt.bfloat16, tag="XG")
            XGr = XG[:].rearrange("p (t m b) -> p t m b", t=T, m=16, b=BL)
            # h sequences, bf16, one tile per dir; free layout (k-section, col)
            # fwd: h_t at col 8*(t+1) (zeros at 0:8); bwd: h_t at col 8*t
            # (zeros at 8T:8(T+1)).
            HF = wp.tile([128, 2 * HC], mybir.dt.bfloat16, tag="HF")
            HB = wp.tile([128, 2 * HC], mybir.dt.bfloat16, tag="HB")
            HFr = HF[:].rearrange("p (k c) -> p k c", k=2)
            HBr = HB[:].rearrange("p (k c) -> p k c", k=2)
            nc.vector.memset(HFr[:, :, 0:BL], 0.0)
            nc.vector.memset(HBr[:, :, BL * T:BL * (T + 1)], 0.0)
            CF = wp.tile([128, 16], mybir.dt.float32, tag="CF")
            CB = wp.tile([128, 16], mybir.dt.float32, tag="CB")
            nc.vector.memset(CF[:], 0.0)
            nc.vector.memset(CB[:], 0.0)

            # ---- input projections ----
            NCH = ROWS // 512
            chunk_order = [0, NCH - 1] + list(range(1, NCH - 1))
            for m in range(16):
                for c in chunk_order:
                    ps = psX.tile([128, 512], mybir.dt.float32)
                    for ki, (k0, kn) in enumerate(KT):
                        nc.tensor.matmul(
                            ps[:],
                            wih[ki][:, 128 * m:128 * (m + 1)],
                            xk[ki][:, 512 * c:512 * (c + 1)],
                            start=(ki == 0), stop=(ki == len(KT) - 1),
                        )
                    tpc = 512 // BL
                    nc.vector.tensor_copy(
                        XGr[:, tpc * c:tpc * (c + 1), m, :],
                        ps[:].rearrange("p (t b) -> p t b", b=BL),
                    )

            # ---- recurrence (fwd and bwd as independent chains) ----
            for t in range(T):
                for d in range(2):
                    td = t if d == 0 else T - 1 - t
                    Hr = HFr if d == 0 else HBr
                    Cst = CF if d == 0 else CB
                    src = BL * td if d == 0 else BL * (td + 1)
                    dst = BL * (td + 1) if d == 0 else BL * td
                    ps = psR.tile([128, 64], mybir.dt.float32)
                    for g in range(8):
                        for k in range(2):
                            nc.tensor.matmul(
                                ps[:, 8 * g:8 * (g + 1)],
                                whh[k][:, G * d + 128 * g:
                                       G * d + 128 * (g + 1)],
                                Hr[:, k, src:src + BL],
                                start=(k == 0), stop=(k == 1),
                            )
                    S = gact.tile([128, 64], mybir.dt.float32)
                    nc.vector.tensor_add(
                        S[:].rearrange("p (m b) -> p m b", b=BL),
                        ps[:].rearrange("p (m b) -> p m b", b=BL),
                        XGr[:, td, 8 * d:8 * (d + 1), :],
                    )
                    nc.scalar.activation(S[:, 0:48], S[:, 0:48], ACT.Sigmoid)
                    nc.scalar.activation(S[:, 48:64], S[:, 48:64], ACT.Tanh)
                    t1 = tmpp.tile([128, 16], mybir.dt.float32, tag="t1")
                    t2 = tmpp.tile([128, 16], mybir.dt.float32, tag="t2")
                    nc.vector.tensor_mul(t1[:], S[:, 0:16], S[:, 48:64])
                    nc.vector.tensor_mul(t2[:], S[:, 16:32], Cst[:])
                    nc.vector.tensor_add(Cst[:], t1[:], t2[:])
                    th = tmpp.tile([128, 16], mybir.dt.float32, tag="th")
                    nc.scalar.activation(th[:], Cst[:], ACT.Tanh)
                    hf32 = tmpp.tile([128, 16], mybir.dt.float32, tag="hf32")
                    nc.vector.tensor_mul(hf32[:], S[:, 32:48], th[:])
                    nc.scalar.copy(
                        Hr[:, :, dst:dst + BL],
                        hf32[:].rearrange("p (k b) -> p k b", k=2),
                    )

            # ---- emissions ----
            EM = wp.tile([K, ROWS], mybir.dt.float32, tag="EM")
            for c in range(NCH):
                pe = psE.tile([K, 512], mybir.dt.float32)
                rhs = [
                    HFr[:, 0, BL + 512 * c:BL + 512 * (c + 1)],
                    HFr[:, 1, BL + 512 * c:BL + 512 * (c + 1)],
                    HBr[:, 0, 512 * c:512 * (c + 1)],
                    HBr[:, 1, 512 * c:512 * (c + 1)],
                ]
                for ki in range(4):
                    nc.tensor.matmul(pe[:], wtag[ki][:], rhs[ki],
                                     start=(ki == 0), stop=(ki == 3))
                nc.vector.tensor_copy(EM[:, 512 * c:512 * (c + 1)], pe[:])
            nc.gpsimd.dma_start(emT[:, :], EM[:])
    nc.compile()
    return nc


def _make_runner(nc):
    """Build a persistent jitted SPMD dispatcher for `nc`.

    run_bass_kernel_spmd re-creates (and re-jits) its shard_map closure on
    every call, costing ~1-2 s of XLA retrace/compile per invocation. This
    builds the same program once and caches the jitted callable, so repeat
    calls pay only host->device transfer + execution.
    """
    import jax
    from jax.sharding import Mesh, PartitionSpec
    from jax.experimental.shard_map import shard_map
    from concourse import bass2jax

    bass2jax.install_neuronx_cc_hook()
    partition_name = (nc.partition_id_tensor.name
                      if nc.partition_id_tensor else None)
    in_names, out_names, out_avals, zero_shapes = [], [], [], []
    for alloc in nc.m.functions[0].allocations:
        if not isinstance(alloc, mybir.MemoryLocationSet):
            continue
        name = alloc.memorylocations[0].name
        if alloc.kind == "ExternalInput":
            if name != partition_name:
                in_names.append(name)
        elif alloc.kind == "ExternalOutput":
            shape = tuple(alloc.tensor_shape)
            dtype = mybir.dt.np(alloc.dtype)
            out_names.append(name)
            out_avals.append(jax.core.ShapedArray(shape, dtype))
            zero_shapes.append((shape, dtype))
    n_params = len(in_names)
    all_names = list(in_names) + list(out_names)
    if partition_name is not None:
        all_names.append(partition_name)
    donate = tuple(range(n_params, n_params + len(out_names)))

    def _body(*args):
        operands = list(args)
        if partition_name is not None:
            operands.append(bass2jax.partition_id_tensor())
        outs = bass2jax._bass_exec_p.bind(
            *operands,
            out_avals=tuple(out_avals),
            in_names=tuple(all_names),
            out_names=tuple(out_names),
            lowering_input_output_aliases=(),
            sim_require_finite=True,
            sim_require_nnan=True,
            nc=nc,
        )
        return tuple(outs)

    devices = jax.devices()[:N_CORES]
    mesh = Mesh(np.asarray(devices), ("core",))
    # Inputs identical across cores (weights) are passed replicated — one
    # host->device copy instead of 8 concatenated shards over the tunnel.
    replicated = {"wihT", "whhT", "wtagT"}
    in_specs = tuple(PartitionSpec() if n in replicated
                     else PartitionSpec("core") for n in in_names)
    in_specs += (PartitionSpec("core"),) * len(out_names)
    sharded = jax.jit(
        shard_map(_body, mesh=mesh, in_specs=in_specs,
                  out_specs=(PartitionSpec("core"),) * len(out_names),
                  check_rep=False),
        donate_argnums=donate, keep_unused=True,
    )

    from jax.sharding import NamedSharding
    dev_cache = {}   # name -> (src np array, on-device jax array)

    def run(in_maps):
        ins = []
        for n in in_names:
            if n in replicated:
                src = np.asarray(in_maps[0][n])
                hit = dev_cache.get(n)
                if hit is not None and (hit[0] is src
                                        or (hit[0].shape == src.shape
                                            and hit[0].dtype == src.dtype
                                            and np.array_equal(hit[0], src))):
                    ins.append(hit[1])
                    continue
                arr = jax.device_put(src, NamedSharding(mesh, PartitionSpec()))
                dev_cache[n] = (src, arr)
                ins.append(arr)
            else:
                ins.append(np.concatenate(
                    [np.asarray(in_maps[c][n]) for c in range(N_CORES)],
                    axis=0))
        concat_zeros = [np.zeros((N_CORES * s[0], *s[1:]), d)
                        for s, d in zero_shapes]
        out_arrs = sharded(*ins, *concat_zeros)
        return [
            {n: np.asarray(out_arrs[i]).reshape(N_CORES, *zero_shapes[i][0])[c]
             for i, n in enumerate(out_names)}
            for c in range(N_CORES)
        ]
    return run


def _run_spmd(nc, in_maps):
    if _CACHE.get("runner_nc") is not nc:
        _CACHE["runner"] = _make_runner(nc)
        _CACHE["runner_nc"] = nc
    return _CACHE["runner"](in_maps)


def _sigmoid(x):
    return 1.0 / (1.0 + np.exp(-x))


def _lstm_dir_from_xg(xg, Whh):
    Bs, Ts, Gd = xg.shape
    Hd = Gd // 4
    WhhT = np.ascontiguousarray(Whh.T)
    h = np.zeros((Bs, Hd), F32)
    c = np.zeros((Bs, Hd), F32)
    out = np.empty((Bs, Ts, Hd), F32)
    for t in range(Ts):
        g = xg[:, t] + h @ WhhT
        i = _sigmoid(g[:, :Hd])
        f = _sigmoid(g[:, Hd:2 * Hd])
        gg = np.tanh(g[:, 2 * Hd:3 * Hd])
        o = _sigmoid(g[:, 3 * Hd:])
        c = f * c + i * gg
        h = o * np.tanh(c)
        out[:, t] = h
    return out


def _lstm_dir_host(x, Wih, Whh, b):
    xg = np.einsum('bti,gi->btg', x, Wih, optimize=True) + b
    return _lstm_dir_from_xg(xg.astype(F32), Whh)


def _logsumexp(a, axis):
    m = np.max(a, axis=axis, keepdims=True)
    return (m + np.log(np.sum(np.exp(a - m), axis=axis,
                              keepdims=True))).squeeze(axis)


def _emissions_host(x, wWih_f, wWhh_f, wb_f, wWih_b, wWhh_b, wb_b, Wtag):
    """fp32 fallback if the device path fails."""
    hf = _lstm_dir_host(x, wWih_f, wWhh_f, wb_f)
    hb = _lstm_dir_host(x[:, ::-1], wWih_b, wWhh_b, wb_b)[:, ::-1]
    seq = np.concatenate([hf, hb], axis=2)
    return np.einsum('bth,kh->btk', seq, Wtag, optimize=True)


def _emissions_device(x, wWih_f, wWhh_f, wb_f, wWih_b, wWhh_b, wb_b, Wtag):
    if "nc" not in _CACHE:
        _CACHE["nc"] = _build_nc()
    nc = _CACHE["nc"]
    wihT = np.ascontiguousarray(np.concatenate([
        np.concatenate([wWih_f, wb_f[:, None]], 1)[PERM].T,
        np.concatenate([wWih_b, wb_b[:, None]], 1)[PERM].T,
    ], axis=1)).astype(BF16)
    whhT = np.ascontiguousarray(
        np.concatenate([wWhh_f[PERM].T, wWhh_b[PERM].T], 1)).astype(BF16)
    wtagT = np.ascontiguousarray(Wtag.T).astype(BF16)
    in_maps = []
    for ci in range(N_CORES):
        xs = x[ci * BL:(ci + 1) * BL]                      # (BL, T, 320)
        xa = np.concatenate([xs, np.ones((BL, T, 1), F32)], 2)
        xTc = np.ascontiguousarray(
            xa.transpose(1, 0, 2).reshape(ROWS, KIN).T).astype(BF16)
        in_maps.append({"xT": xTc, "wihT": wihT, "whhT": whhT,
                        "wtagT": wtagT})
    _CACHE["last_in_maps"] = in_maps
    # First exec on a freshly-compiled NEFF occasionally hits a transient
    # NRT_EXEC_UNIT_UNRECOVERABLE on this axon tunnel; retry (with a fresh
    # build on the second failure).
    results = None
    for attempt in range(3):
        try:
            results = _run_spmd(nc, in_maps)
            break
        except Exception:
            if attempt == 2:
                raise
            import time as _time
            _time.sleep(5)
            if attempt == 1:
                _CACHE.pop("nc", None)
                _CACHE.pop("runner_nc", None)
                nc = _CACHE.setdefault("nc", _build_nc())
    em = np.empty((B, T, K), F32)
    for ci in range(N_CORES):
        emc = results[ci]["emT"]                           # (K, ROWS)
        em[ci * BL:(ci + 1) * BL] = emc.T.reshape(T, BL, K).transpose(1, 0, 2)
    return em


def kernel(char_tensor, token_tensor, tags, mask, emb,
           cWih_f, cWhh_f, cb_f, cWih_b, cWhh_b, cb_b,
           wWih_f, wWhh_f, wb_f, wWih_b, wWhh_b, wb_b,
           Wtag, btag, start_t, end_t, trans):
    f32 = lambda a: np.asarray(a, F32)
    char_tensor = f32(char_tensor)
    emb = f32(emb)
    token_tensor = np.asarray(token_tensor).astype(np.int64)
    tags_i = np.asarray(tags).astype(np.int64)
    mask_b = np.asarray(mask).astype(bool)

    # --- char BiLSTM (tiny) + embedding gather on host ---
    cf = _lstm_dir_host(char_tensor, f32(cWih_f), f32(cWhh_f), f32(cb_f))
    cb = _lstm_dir_host(char_tensor[:, ::-1], f32(cWih_b), f32(cWhh_b),
                        f32(cb_b))[:, ::-1]
    word_emb = emb[token_tensor]                                  # (B,T,300)
    x = np.concatenate([cf, cb, word_emb], axis=2)                # (B,T,320)

    # --- word BiLSTM + emissions on the 8 NeuronCores ---
    args = (x, f32(wWih_f), f32(wWhh_f), f32(wb_f), f32(wWih_b),
            f32(wWhh_b), f32(wb_b), f32(Wtag))
    try:
        em = _emissions_device(*args)
    except Exception:
        em = _emissions_host(*args)
    em = em + f32(btag)

    # --- CRF NLL on host ---
    em = np.swapaxes(em, 0, 1)                                    # (T,B,K)
    tg = np.swapaxes(tags_i, 0, 1)
    m = np.swapaxes(mask_b, 0, 1).astype(F32)
    start_t, end_t, trans = f32(start_t), f32(end_t), f32(trans)
    bidx = np.arange(B)
    e_sc = np.take_along_axis(em, tg[..., None], axis=-1)[..., 0]  # (T,B)
    num = start_t[tg[0]] + e_sc[0]
    num = num + np.sum((trans[tg[:-1], tg[1:]] + e_sc[1:]) * m[1:], axis=0)
    last = (np.sum(m, axis=0) - 1).astype(np.int64)
    num = num + end_t[tg[last, bidx]]
    alpha = start_t[None, :] + em[0]
    for t in range(1, T):
        nxt = _logsumexp(alpha[:, :, None] + trans[None, :, :]
                         + em[t][:, None, :], axis=1)
        alpha = np.where(m[t][:, None] > 0, nxt, alpha)
    den = _logsumexp(alpha + end_t[None, :], axis=1)
    return F32(-np.sum(num - den))
